# revision 22
# baseline (speedup 1.0000x reference)
"""Trainium2 Bass kernel for AsymmetricQuantLinear (int4 asymmetric dequant + matmul).

    x:             [4096, 4096]  f32
    weight_packed: [2048, 11008] int32 (two 4-bit nibbles per value, only low byte used)
    weight_scale:  [11008] f32
    weight_zero:   [11008] f32
    out = x @ ((unpack(weight_packed) - zero) * scale)   -> [4096, 11008] f32

Strategy: tensor-parallel over N across 8 NeuronCores (1376 output cols each),
x replicated. Host does layout-only prep: x -> bf16, transposed to K-major;
int4 nibbles widened to bf16 rows in natural K order; scale/zero replicated
across 128 partitions. Device dequantizes W to bf16 on DVE (two 16-bit ops per
k-tile) and runs the bf16 matmul on the TensorEngine with fp32 PSUM
accumulation; the first m-block interleaves both its m-subtiles across one
k-sweep (6 PSUM banks) so PE stays fed while dequant streams in.
"""

import numpy as np
import ml_dtypes

M, K, N = 4096, 4096, 11008
N_CORES = 8
N_SHARD = N // N_CORES          # 1376
P = 128
KT = K // P                     # 32 k-tiles
MSW = 256                       # m columns fetched per x DMA (two 128-wide m-tiles)
MSUP = M // MSW                 # 16
N_CHUNKS = [(0, 512), (512, 512), (1024, 352)]

_compiled = None


def _build():
    import concourse.mybir as mybir
    import concourse.tile as tile
    from concourse import bacc

    f32 = mybir.dt.float32
    bf16 = mybir.dt.bfloat16
    u8 = mybir.dt.uint8

    nc = bacc.Bacc("TRN2", target_bir_lowering=False, debug=False, num_devices=N_CORES)
    xt = nc.dram_tensor("xt", [K, M], bf16, kind="ExternalInput").ap()
    q = nc.dram_tensor("q", [K, N_SHARD], bf16, kind="ExternalInput").ap()
    z = nc.dram_tensor("z", [P, N_SHARD], bf16, kind="ExternalInput").ap()
    s = nc.dram_tensor("s", [P, N_SHARD], bf16, kind="ExternalInput").ap()
    out = nc.dram_tensor("out", [M, N_SHARD], f32, kind="ExternalOutput").ap()

    with tile.TileContext(nc) as tc:
        with (
            tc.tile_pool(name="const", bufs=1) as constp,
            tc.tile_pool(name="wq", bufs=1) as wqp,
            tc.tile_pool(name="qstage", bufs=6) as qp,
            tc.tile_pool(name="xin", bufs=3) as xp,
            tc.tile_pool(name="ostage", bufs=3) as outp,
            tc.tile_pool(name="psum", space="PSUM", bufs=2) as pp,
        ):

            # Dequant: W[kt] = (q[kt] - zero) * scale, all-bf16 DVE ops, W
            # SBUF-resident. The first x block's DMA is issued after two
            # k-tiles so it overlaps the rest of the dequant stream.
            q3 = q.rearrange("(kt p) n -> p kt n", p=P)
            xt3 = xt.rearrange("(kt p) m -> p kt m", p=P)
            wq_tiles = []

            # First x slices and the dequant constants lead the DMA stream so
            # the first matmul can issue as early as possible.
            x0_t = xp.tile([P, KT, MSW], bf16, tag="x", name="x_t")
            x1_t = xp.tile([P, KT, MSW], bf16, tag="x", name="x_t")
            nc.sync.dma_start(x0_t[:, 0:4, :], xt3[:, 0:4, 0:MSW])
            nc.sync.dma_start(x1_t[:, 0:4, :], xt3[:, 0:4, MSW:2 * MSW])
            z_t = constp.tile([P, N_SHARD], bf16, tag="z")
            nc.sync.dma_start(z_t[:], z[:])
            s_t = constp.tile([P, N_SHARD], bf16, tag="s")
            nc.sync.dma_start(s_t[:], s[:])

            def dequant(kt):
                qt = qp.tile([P, N_SHARD], bf16, tag="qt", name="qt")
                nc.sync.dma_start(qt[:], q3[:, kt, :])
                wt = wqp.tile([P, N_SHARD], bf16, tag=f"wq{kt}", name="wt")
                nc.vector.tensor_sub(wt[:], qt[:], z_t[:])
                nc.vector.tensor_mul(wt[:], wt[:], s_t[:])
                wq_tiles.append(wt)

            for kt in range(2):
                dequant(kt)
            # x0/x1 are demanded k-tile by k-tile (msi 0 and 1 interleave),
            # so their transfers are sliced into 4-ktile pieces woven into the
            # q stream: supply tracks demand instead of two 2MB stalls.
            for kt in range(2, KT):
                dequant(kt)
                if kt % 4 == 1:
                    g = (kt - 1) // 4
                    nc.sync.dma_start(
                        x0_t[:, 4 * g:4 * g + 4, :], xt3[:, 4 * g:4 * g + 4, 0:MSW])
                    nc.sync.dma_start(
                        x1_t[:, 4 * g:4 * g + 4, :], xt3[:, 4 * g:4 * g + 4, MSW:2 * MSW])

            # Matmul: out[m-tile] accumulates over all 32 k-tiles. x blocks
            # are prefetched one msi ahead so their DMAs issue before this
            # msi's output DMAs block the sync queue.
            for msi in range(MSUP):
                if msi == 0:
                    x_t = x0_t
                elif msi == 1:
                    x_t = x1_t
                else:
                    x_t = xp.tile([P, KT, MSW], bf16, tag="x", name="x_t")
                    nc.sync.dma_start(x_t[:], xt3[:, :, msi * MSW:(msi + 1) * MSW])
                subs = range(MSW // P)
                if msi <= 1:
                    # Both m-subtiles interleaved in one k-sweep: PE has 6 MMs
                    # of work per arriving dequant tile instead of 3.
                    o_ts = [outp.tile([P, N_SHARD], f32, tag="o", name="o_t") for _ in subs]
                    psss = [
                        [
                            pp.tile([P, nw], f32, tag=f"ps{ci}", name=f"ps{ci}")
                            for ci, (n0, nw) in enumerate(N_CHUNKS)
                        ]
                        for _ in subs
                    ]
                    for kt in range(KT):
                        for sub in subs:
                            lhsT = x_t[:, kt, sub * P:(sub + 1) * P]
                            for ci, (n0, nw) in enumerate(N_CHUNKS):
                                nc.tensor.matmul(
                                    psss[sub][ci][:],
                                    lhsT,
                                    wq_tiles[kt][:, n0:n0 + nw],
                                    start=(kt == 0),
                                    stop=(kt == KT - 1),
                                )
                    for sub in subs:
                        for ci, (n0, nw) in enumerate(N_CHUNKS):
                            nc.scalar.copy(o_ts[sub][:, n0:n0 + nw], psss[sub][ci][:])
                        m0 = (msi * (MSW // P) + sub) * P
                        nc.sync.dma_start(out[m0:m0 + P, :], o_ts[sub][:])
                    continue
                for sub in subs:
                    o_t = outp.tile([P, N_SHARD], f32, tag="o")
                    pss = [
                        pp.tile([P, nw], f32, tag=f"ps{ci}", name=f"ps{ci}")
                        for ci, (n0, nw) in enumerate(N_CHUNKS)
                    ]
                    for kt in range(KT):
                        lhsT = x_t[:, kt, sub * P:(sub + 1) * P]
                        for ci, (n0, nw) in enumerate(N_CHUNKS):
                            nc.tensor.matmul(
                                pss[ci][:],
                                lhsT,
                                wq_tiles[kt][:, n0:n0 + nw],
                                start=(kt == 0),
                                stop=(kt == KT - 1),
                            )
                    for ci, (n0, nw) in enumerate(N_CHUNKS):
                        nc.scalar.copy(o_t[:, n0:n0 + nw], pss[ci][:])
                    m0 = (msi * (MSW // P) + sub) * P
                    nc.sync.dma_start(out[m0:m0 + P, :], o_t[:])

    nc.compile()
    return nc


def _prep_in_maps(x, weight_packed, weight_scale, weight_zero):
    x = np.asarray(x, dtype=np.float32)
    wp = np.asarray(weight_packed, dtype=np.int32)
    ws = np.asarray(weight_scale, dtype=np.float32)
    wz = np.asarray(weight_zero, dtype=np.float32)

    xt = np.ascontiguousarray(x.T).astype(ml_dtypes.bfloat16)  # [K, M] bf16
    qfull = np.empty((K, N), dtype=ml_dtypes.bfloat16)
    qfull[0::2] = (wp & 15).astype(ml_dtypes.bfloat16)
    qfull[1::2] = ((wp >> 4) & 15).astype(ml_dtypes.bfloat16)
    zb = wz.astype(ml_dtypes.bfloat16)
    sb = ws.astype(ml_dtypes.bfloat16)

    in_maps = []
    for c in range(N_CORES):
        n0, n1 = c * N_SHARD, (c + 1) * N_SHARD
        in_maps.append({
            "xt": xt,
            "q": np.ascontiguousarray(qfull[:, n0:n1]),
            "z": np.ascontiguousarray(np.broadcast_to(zb[n0:n1][None, :], (P, N_SHARD))),
            "s": np.ascontiguousarray(np.broadcast_to(sb[n0:n1][None, :], (P, N_SHARD))),
        })
    return in_maps


def run(x, weight_packed, weight_scale, weight_zero, trace=False, **spmd_kwargs):
    from concourse.bass_utils import run_bass_kernel_spmd

    global _compiled
    if _compiled is None:
        _compiled = _build()
    in_maps = _prep_in_maps(x, weight_packed, weight_scale, weight_zero)
    res = run_bass_kernel_spmd(
        _compiled, in_maps, core_ids=list(range(N_CORES)), trace=trace, **spmd_kwargs
    )
    full = np.concatenate([res.results[c]["out"] for c in range(N_CORES)], axis=1)
    return full, res


def kernel(x, weight_packed, weight_scale, weight_zero):
    full, _ = run(x, weight_packed, weight_scale, weight_zero, trace=False)
    return full


# revision 23
# speedup vs baseline: 1.1941x; 1.1941x over previous
"""Trainium2 Bass kernel for AsymmetricQuantLinear (int4 asymmetric dequant + matmul).

    x:             [4096, 4096]  f32
    weight_packed: [2048, 11008] int32 (two 4-bit nibbles per value, only low byte used)
    weight_scale:  [11008] f32
    weight_zero:   [11008] f32
    out = x @ ((unpack(weight_packed) - zero) * scale)   -> [4096, 11008] f32

Strategy: tensor-parallel over N across 8 NeuronCores (1376 output cols each),
x replicated. Host does layout-only prep: x -> bf16, transposed to K-major;
int4 nibbles widened to bf16 rows in natural K order; scale/zero replicated
across 128 partitions. Device dequantizes W to bf16 on DVE (two 16-bit ops per
k-tile) and runs the bf16 matmul on the TensorEngine with fp32 PSUM
accumulation; the first m-block interleaves both its m-subtiles across one
k-sweep (6 PSUM banks) so PE stays fed while dequant streams in.
"""

import numpy as np
import ml_dtypes

M, K, N = 4096, 4096, 11008
N_CORES = 8
N_SHARD = N // N_CORES          # 1376
P = 128
KT = K // P                     # 32 k-tiles
MSW = 256                       # m columns fetched per x DMA (two 128-wide m-tiles)
MSUP = M // MSW                 # 16
N_CHUNKS = [(0, 512), (512, 512), (1024, 352)]

_compiled = None


def _build():
    import concourse.mybir as mybir
    import concourse.tile as tile
    from concourse import bacc

    f32 = mybir.dt.float32
    bf16 = mybir.dt.bfloat16
    u8 = mybir.dt.uint8

    nc = bacc.Bacc("TRN2", target_bir_lowering=False, debug=False, num_devices=N_CORES)
    xt = nc.dram_tensor("xt", [K, M], bf16, kind="ExternalInput").ap()
    q = nc.dram_tensor("q", [K, N_SHARD], bf16, kind="ExternalInput").ap()
    z = nc.dram_tensor("z", [P, N_SHARD], bf16, kind="ExternalInput").ap()
    s = nc.dram_tensor("s", [P, N_SHARD], bf16, kind="ExternalInput").ap()
    out = nc.dram_tensor("out", [M, N_SHARD], f32, kind="ExternalOutput").ap()

    with tile.TileContext(nc) as tc:
        with (
            tc.tile_pool(name="const", bufs=1) as constp,
            tc.tile_pool(name="wq", bufs=1) as wqp,
            tc.tile_pool(name="qstage", bufs=6) as qp,
            tc.tile_pool(name="xin", bufs=3) as xp,
            tc.tile_pool(name="ostage", bufs=3) as outp,
            tc.tile_pool(name="psum", space="PSUM", bufs=2) as pp,
        ):

            # Dequant: W[kt] = (q[kt] - zero) * scale, all-bf16 DVE ops, W
            # SBUF-resident. The first x block's DMA is issued after two
            # k-tiles so it overlaps the rest of the dequant stream.
            q3 = q.rearrange("(kt p) n -> p kt n", p=P)
            xt3 = xt.rearrange("(kt p) m -> p kt m", p=P)
            wq_tiles = []

            z_t = constp.tile([P, N_SHARD], bf16, tag="z")
            nc.sync.dma_start(z_t[:], z[:])
            s_t = constp.tile([P, N_SHARD], bf16, tag="s")
            nc.sync.dma_start(s_t[:], s[:])

            def dequant(kt):
                qt = qp.tile([P, N_SHARD], bf16, tag="qt", name="qt")
                nc.sync.dma_start(qt[:], q3[:, kt, :])
                wt = wqp.tile([P, N_SHARD], bf16, tag=f"wq{kt}", name="wt")
                nc.vector.tensor_sub(wt[:], qt[:], z_t[:])
                nc.vector.tensor_mul(wt[:], wt[:], s_t[:])
                wq_tiles.append(wt)

            for kt in range(2):
                dequant(kt)
            # x0/x1 are demanded k-tile by k-tile (msi 0 and 1 interleave),
            # so their transfers are sliced into 4-ktile pieces woven into the
            # q stream: supply tracks demand instead of two 2MB stalls.
            x0_t = xp.tile([P, KT, MSW], bf16, tag="x", name="x_t")
            x1_t = xp.tile([P, KT, MSW], bf16, tag="x", name="x_t")
            nc.sync.dma_start(x0_t[:, 0:4, :], xt3[:, 0:4, 0:MSW])
            nc.sync.dma_start(x1_t[:, 0:4, :], xt3[:, 0:4, MSW:2 * MSW])
            for kt in range(2, KT):
                dequant(kt)
                if kt % 4 == 1:
                    g = (kt - 1) // 4
                    nc.sync.dma_start(
                        x0_t[:, 4 * g:4 * g + 4, :], xt3[:, 4 * g:4 * g + 4, 0:MSW])
                    nc.sync.dma_start(
                        x1_t[:, 4 * g:4 * g + 4, :], xt3[:, 4 * g:4 * g + 4, MSW:2 * MSW])

            # Matmul: out[m-tile] accumulates over all 32 k-tiles. x blocks
            # are prefetched one msi ahead so their DMAs issue before this
            # msi's output DMAs block the sync queue.
            for msi in range(MSUP):
                if msi == 0:
                    x_t = x0_t
                elif msi == 1:
                    x_t = x1_t
                else:
                    x_t = xp.tile([P, KT, MSW], bf16, tag="x", name="x_t")
                    nc.sync.dma_start(x_t[:], xt3[:, :, msi * MSW:(msi + 1) * MSW])
                subs = range(MSW // P)
                if msi <= 1:
                    # Both m-subtiles interleaved in one k-sweep: PE has 6 MMs
                    # of work per arriving dequant tile instead of 3.
                    o_ts = [outp.tile([P, N_SHARD], f32, tag="o", name="o_t") for _ in subs]
                    psss = [
                        [
                            pp.tile([P, nw], f32, tag=f"ps{ci}", name=f"ps{ci}")
                            for ci, (n0, nw) in enumerate(N_CHUNKS)
                        ]
                        for _ in subs
                    ]
                    for kt in range(KT):
                        for sub in subs:
                            lhsT = x_t[:, kt, sub * P:(sub + 1) * P]
                            for ci, (n0, nw) in enumerate(N_CHUNKS):
                                nc.tensor.matmul(
                                    psss[sub][ci][:],
                                    lhsT,
                                    wq_tiles[kt][:, n0:n0 + nw],
                                    start=(kt == 0),
                                    stop=(kt == KT - 1),
                                )
                    for sub in subs:
                        for ci, (n0, nw) in enumerate(N_CHUNKS):
                            nc.scalar.copy(o_ts[sub][:, n0:n0 + nw], psss[sub][ci][:])
                        m0 = (msi * (MSW // P) + sub) * P
                        nc.sync.dma_start(out[m0:m0 + P, :], o_ts[sub][:])
                    continue
                for sub in subs:
                    o_t = outp.tile([P, N_SHARD], f32, tag="o")
                    pss = [
                        pp.tile([P, nw], f32, tag=f"ps{ci}", name=f"ps{ci}")
                        for ci, (n0, nw) in enumerate(N_CHUNKS)
                    ]
                    for kt in range(KT):
                        lhsT = x_t[:, kt, sub * P:(sub + 1) * P]
                        for ci, (n0, nw) in enumerate(N_CHUNKS):
                            nc.tensor.matmul(
                                pss[ci][:],
                                lhsT,
                                wq_tiles[kt][:, n0:n0 + nw],
                                start=(kt == 0),
                                stop=(kt == KT - 1),
                            )
                    for ci, (n0, nw) in enumerate(N_CHUNKS):
                        nc.scalar.copy(o_t[:, n0:n0 + nw], pss[ci][:])
                    m0 = (msi * (MSW // P) + sub) * P
                    nc.sync.dma_start(out[m0:m0 + P, :], o_t[:])

    nc.compile()
    return nc


def _prep_in_maps(x, weight_packed, weight_scale, weight_zero):
    x = np.asarray(x, dtype=np.float32)
    wp = np.asarray(weight_packed, dtype=np.int32)
    ws = np.asarray(weight_scale, dtype=np.float32)
    wz = np.asarray(weight_zero, dtype=np.float32)

    xt = np.ascontiguousarray(x.T).astype(ml_dtypes.bfloat16)  # [K, M] bf16
    qfull = np.empty((K, N), dtype=ml_dtypes.bfloat16)
    qfull[0::2] = (wp & 15).astype(ml_dtypes.bfloat16)
    qfull[1::2] = ((wp >> 4) & 15).astype(ml_dtypes.bfloat16)
    zb = wz.astype(ml_dtypes.bfloat16)
    sb = ws.astype(ml_dtypes.bfloat16)

    in_maps = []
    for c in range(N_CORES):
        n0, n1 = c * N_SHARD, (c + 1) * N_SHARD
        in_maps.append({
            "xt": xt,
            "q": np.ascontiguousarray(qfull[:, n0:n1]),
            "z": np.ascontiguousarray(np.broadcast_to(zb[n0:n1][None, :], (P, N_SHARD))),
            "s": np.ascontiguousarray(np.broadcast_to(sb[n0:n1][None, :], (P, N_SHARD))),
        })
    return in_maps


def run(x, weight_packed, weight_scale, weight_zero, trace=False, **spmd_kwargs):
    from concourse.bass_utils import run_bass_kernel_spmd

    global _compiled
    if _compiled is None:
        _compiled = _build()
    in_maps = _prep_in_maps(x, weight_packed, weight_scale, weight_zero)
    res = run_bass_kernel_spmd(
        _compiled, in_maps, core_ids=list(range(N_CORES)), trace=trace, **spmd_kwargs
    )
    full = np.concatenate([res.results[c]["out"] for c in range(N_CORES)], axis=1)
    return full, res


def kernel(x, weight_packed, weight_scale, weight_zero):
    full, _ = run(x, weight_packed, weight_scale, weight_zero, trace=False)
    return full
